# revision 24
# baseline (speedup 1.0000x reference)
"""Trainium2 Bass kernel for attention-weight computation.

Computes attn = softmax(encoder_outputs @ hidden) over seq_len=65536,
returning shape (1, 1, 65536) float32.

Distribution: encoder_outputs [65536, 1024] is sharded by rows across 8
NeuronCores (8192 rows each).  The host hands each core the TRANSPOSE of
its slice ([1024, 8192], h-major) so the contraction dim (h) lies on SBUF
partitions; the core streams it from HBM in 2 MiB tiles and computes its
8192 scores on the TensorEngine (hidden chunk = 1-column stationary
operand, E^T tile = [128, 512] moving operand, accumulating over the 8
h-chunks in [1, 512] PSUM tiles).

Softmax uses a single AllGather (flash-softmax style): each core reshapes
its scores to [16, 512] via a 32 KiB DRAM round-trip, computes
per-partition (max m_t, sum_t exp(s - m_t)) pairs, AllGathers the 8x32
pair vectors, locally combines them into the global max g and global sum
S, and rescales: attn = exp(s - m_t) * exp(m_t - g) / S.
"""

import numpy as np

S_TOTAL = 65536
H = 1024
N_CORES = 8
S_PER = S_TOTAL // N_CORES  # 8192 rows per core
P = 128                     # SBUF partitions
HC = H // P                 # 8 h-chunks
S_CHUNK = 512               # s extent of one streaming DMA tile (2 MiB)
N_SC = S_PER // S_CHUNK     # 16 s-chunks == 16 score segments
SEG = S_CHUNK               # segment width in the [N_SC, SEG] score layout
NSTREAM = 4                 # concurrent PE column-group streams
NROUND = N_SC // NSTREAM    # 4 rounds of 4 streams

_CACHE: dict = {}


def _build_module(mm_dtype: str = "float32"):
    import concourse.bacc as bacc
    import concourse.mybir as mybir
    import concourse.tile as tile

    fp32 = mybir.dt.float32
    mmdt = getattr(mybir.dt, mm_dtype)
    AX = mybir.AxisListType.X
    ALL_CORES = [list(range(N_CORES))]
    Act = mybir.ActivationFunctionType

    nc = bacc.Bacc(
        "TRN2",
        target_bir_lowering=False,
        debug=False,
        enable_asserts=False,
        num_devices=N_CORES,
    )

    # et: transposed slice [H, S_PER]; hc: hidden as [P, HC] (chunk j in col j)
    et = nc.dram_tensor("et", [H, S_PER], mmdt, kind="ExternalInput").ap()
    hc = nc.dram_tensor("hc", [P, HC], mmdt, kind="ExternalInput").ap()
    out = nc.dram_tensor("out", [S_PER], fp32, kind="ExternalOutput").ap()

    with tile.TileContext(nc) as tc:
        with (
            tc.tile_pool(name="stream", bufs=4) as stream_pool,
            tc.tile_pool(name="persist", bufs=1) as persist_pool,
            tc.tile_pool(name="small", bufs=1) as small_pool,
            tc.tile_pool(name="psum", bufs=1, space="PSUM") as psum_pool,
            tc.tile_pool(name="dram", bufs=1, space="DRAM") as dram_pool,
        ):
            hid = small_pool.tile([P, HC], mmdt)
            nc.sync.dma_start(out=hid, in_=hc)
            ones = small_pool.tile([1, P], fp32)
            nc.vector.memset(ones, 1.0)

            # ---- scores: hidden chunk stationary, E^T moving ----
            # first chunks are small so the PE starts within a few us
            sizes = [128, 128, 256] + [S_CHUNK] * ((S_PER - 512) // S_CHUNK)
            assert sum(sizes) == S_PER
            et3 = et.rearrange("(j p) s -> p j s", p=P)
            scores_row = persist_pool.tile([1, S_PER], fp32)
            offs = [sum(sizes[:i]) for i in range(len(sizes))]

            def load_chunk(c):
                etile = stream_pool.tile(
                    [P, HC, sizes[c]], mmdt, tag="et", bufs=6, name=f"et{c}"
                )
                nc.sync.dma_start(
                    out=etile, in_=et3[:, :, offs[c] : offs[c] + sizes[c]]
                )
                return etile

            # interleave two accumulation chains (distinct PSUM banks) so
            # one chain's weight loads hide under the other's matmuls
            for c0 in range(0, len(sizes), 2):
                eta = load_chunk(c0)
                etb = load_chunk(c0 + 1) if c0 + 1 < len(sizes) else None
                psa = psum_pool.tile(
                    [1, sizes[c0]], fp32, tag="ps", bufs=4, name=f"ps{c0}"
                )
                psb = (
                    psum_pool.tile(
                        [1, sizes[c0 + 1]], fp32, tag="ps", bufs=4, name=f"ps{c0 + 1}"
                    )
                    if etb is not None
                    else None
                )
                for j in range(HC):
                    nc.tensor.matmul(
                        psa,
                        hid[:, j : j + 1],
                        eta[:, j, :],
                        start=(j == 0),
                        stop=(j == HC - 1),
                    )
                    if etb is not None:
                        nc.tensor.matmul(
                            psb,
                            hid[:, j : j + 1],
                            etb[:, j, :],
                            start=(j == 0),
                            stop=(j == HC - 1),
                        )
                nc.scalar.copy(
                    scores_row[:, offs[c0] : offs[c0] + sizes[c0]], psa
                )
                if etb is not None:
                    nc.scalar.copy(
                        scores_row[:, offs[c0 + 1] : offs[c0 + 1] + sizes[c0 + 1]],
                        psb,
                    )

            # ---- reshape scores [1, 8192] -> [16, 512] via DRAM ----
            sc_dram = dram_pool.tile([S_PER], fp32)
            nc.sync.dma_start(out=sc_dram, in_=scores_row)
            scores16 = persist_pool.tile([N_SC, SEG], fp32)
            nc.sync.dma_start(
                out=scores16, in_=sc_dram.rearrange("(t n) -> t n", t=N_SC)
            )

            # ---- local flash-softmax stats ----
            pair16 = small_pool.tile([N_SC, 2], fp32)  # [:,0]=max, [:,1]=sumexp
            nc.vector.reduce_max(pair16[:, 0:1], scores16, axis=AX)
            negm16 = small_pool.tile([N_SC, 1], fp32)
            nc.vector.tensor_scalar_mul(negm16, pair16[:, 0:1], -1.0)
            exps16 = persist_pool.tile([N_SC, SEG], fp32)
            nc.scalar.activation(
                out=exps16,
                in_=scores16,
                func=Act.Exp,
                bias=negm16,
                scale=1.0,
                accum_out=pair16[:, 1:2],
            )

            # ---- one AllGather of the (m, S) pairs ----
            cc_in = dram_pool.tile([N_SC * 2], fp32)
            cc_out = dram_pool.tile([N_CORES, N_SC * 2], fp32)
            nc.sync.dma_start(out=cc_in, in_=pair16)
            nc.gpsimd.collective_compute(
                "AllGather",
                mybir.AluOpType.bypass,
                replica_groups=ALL_CORES,
                ins=[cc_in.opt()],
                outs=[cc_out.opt()],
            )

            # ---- combine: g = max m; S = sum s*exp(m-g); factors ----
            row = small_pool.tile([1, N_CORES * N_SC * 2], fp32)
            nc.sync.dma_start(out=row, in_=cc_out.rearrange("a b -> (a b)"))
            rowv = row.rearrange("o (k two) -> o two k", two=2)
            g1 = small_pool.tile([1, 1], fp32)
            nc.vector.reduce_max(g1, rowv[:, 0, :], axis=AX)
            negg1 = small_pool.tile([1, 1], fp32)
            nc.vector.tensor_scalar_mul(negg1, g1, -1.0)
            em = small_pool.tile([1, N_CORES * N_SC], fp32)
            nc.scalar.activation(
                out=em, in_=rowv[:, 0, :], func=Act.Exp, bias=negg1, scale=1.0
            )
            terms = small_pool.tile([1, N_CORES * N_SC], fp32)
            nc.vector.tensor_mul(terms, em, rowv[:, 1, :])
            s1 = small_pool.tile([1, 1], fp32)
            nc.vector.reduce_sum(s1, terms, axis=AX)
            rs1 = small_pool.tile([1, 1], fp32)
            nc.vector.reciprocal(rs1, s1)

            # broadcast (-g, 1/S) to the 16 partitions via ones.T @ pack
            pack = small_pool.tile([1, 2], fp32)
            nc.vector.tensor_copy(pack[:, 0:1], negg1)
            nc.vector.tensor_copy(pack[:, 1:2], rs1)
            bpsum = psum_pool.tile([N_SC, 2], fp32)
            nc.tensor.matmul(bpsum, ones[:, 0:N_SC], pack, start=True, stop=True)
            bsc = small_pool.tile([N_SC, 2], fp32)
            nc.scalar.copy(bsc, bpsum)

            # f = exp(m - g); attn = exps * f * (1/S)
            f16 = small_pool.tile([N_SC, 1], fp32)
            nc.scalar.activation(
                out=f16, in_=pair16[:, 0:1], func=Act.Exp, bias=bsc[:, 0:1], scale=1.0
            )
            attn16 = persist_pool.tile([N_SC, SEG], fp32)
            nc.vector.tensor_scalar(
                out=attn16,
                in0=exps16,
                scalar1=f16,
                scalar2=bsc[:, 1:2],
                op0=mybir.AluOpType.mult,
                op1=mybir.AluOpType.mult,
            )
            nc.sync.dma_start(
                out=out.rearrange("(t n) -> t n", t=N_SC), in_=attn16
            )

    nc.compile()
    return nc


def _get_module():
    if "nc" not in _CACHE:
        _CACHE["nc"] = _build_module()
    return _CACHE["nc"]


def _prep_inputs(hidden: np.ndarray, encoder_outputs: np.ndarray):
    hidden = np.asarray(hidden, dtype=np.float32)
    eo = np.asarray(encoder_outputs, dtype=np.float32)
    hcm = np.ascontiguousarray(hidden.reshape(HC, P).T)  # [P, HC]
    in_maps = []
    for c in range(N_CORES):
        ets = np.ascontiguousarray(eo[c * S_PER : (c + 1) * S_PER].T)  # [H, S_PER]
        in_maps.append({"et": ets, "hc": hcm})
    return in_maps


def _run(hidden: np.ndarray, encoder_outputs: np.ndarray, trace: bool = False):
    from concourse.bass_utils import run_bass_kernel_spmd

    nc = _get_module()
    in_maps = _prep_inputs(hidden, encoder_outputs)
    res = run_bass_kernel_spmd(
        nc, in_maps, core_ids=list(range(N_CORES)), trace=trace
    )
    parts = [np.asarray(res.results[c]["out"]).reshape(-1) for c in range(N_CORES)]
    attn = np.concatenate(parts)
    return attn.reshape(1, 1, S_TOTAL).astype(np.float32), res


def kernel(hidden: np.ndarray, encoder_outputs: np.ndarray) -> np.ndarray:
    out, _ = _run(hidden, encoder_outputs, trace=False)
    return out
